# revision 2
# baseline (speedup 1.0000x reference)
"""Multi-head causal attention (Whisper-style) on 8 trn2 NeuronCores.

Sharding: hybrid batch x head-quad.  Core c handles batch c//4 and heads
4*(c%4) .. 4*(c%4)+3 (E=256 feature columns of Wq/Wk/Wv, 256 rows of Wo),
producing a full-width partial yT for its batch.  The host sums the 4
partials per batch, transposes back, and adds bo.

On-chip layout is fully transposed (feature dim on partitions), heads
organized in two head-pairs (hp) of 128 dims each:
  qT,kT[hp] = [128, S] per head-pair (SCALE folded into Wq/Wk host-side)
  scores^T[k,q] per (hp, head); softmax along partitions via a ones
  column appended to v (denominator rides the o^T matmul); no
  max-subtraction (exp(-1e9 + s) underflows to exactly 0).
Causal structure is exploited trapezoid-style: mask blocks are
classified host-side; "partial" (diagonal) blocks with column offset d
compute scores/exp/o only on columns [d:512), with the fixed [128,128]
triangle mask tile added in-PSUM on the DVE before exp.  exp output (the
attention weights) and v are bf16; everything else fp32/fp32r.
"""

import os
import sys
from contextlib import ExitStack

import numpy as np

for _p in ("/root/.axon_site/_ro/trn_rl_repo", "/opt/trn_rl_repo"):
    if os.path.isdir(_p) and _p not in sys.path:
        sys.path.append(_p)

import concourse.bass as bass
import concourse.mybir as mybir
import concourse.tile as tile
from concourse import bacc, bass_utils

F32 = mybir.dt.float32
F32R = mybir.dt.float32r
BF16 = mybir.dt.bfloat16
AF = mybir.ActivationFunctionType
ALU = mybir.AluOpType

N_STATE = 1024
N_HEAD = 16
HD = 64
N_CORES = 8
HEADS_PER_CORE = 4
E = HEADS_PER_CORE * HD       # 256 feature columns per core
HP = 2                        # head-pairs per core (128 dims each)
Q_TILE = 512
K_CHUNK = 128
N_D = N_STATE // 128          # 8 contraction chunks for the projections
SCALE = float(HD) ** -0.25
NEG_THRESH = -50.0


def classify_blocks(maskT):
    """Classify (k_chunk, q_tile) mask blocks; find trapezoid offsets.

    Returns (cls, d_off) where cls[(ki,j)] in {skip, clean, partial} and
    d_off[(ki,j)] = first unmasked column of a partial block.  Verifies
    the trapezoid assumption: cols [0:d) all masked, cols [d:d+128) equal
    to the canonical triangle tile maskT[0:128, 0:128], cols [d+128:)
    all zero, and that each j's first non-skip block is full width.
    """
    S = maskT.shape[0]
    tri = maskT[0:K_CHUNK, 0:K_CHUNK]
    assert (np.all(tri[np.tril_indices(K_CHUNK, -1)] < NEG_THRESH)
            and np.all(tri[np.triu_indices(K_CHUNK, 0)] == 0.0)), \
        "unexpected triangle tile"
    cls, d_off = {}, {}
    for ki in range(S // K_CHUNK):
        for j in range(S // Q_TILE):
            blk = maskT[ki * K_CHUNK:(ki + 1) * K_CHUNK,
                        j * Q_TILE:(j + 1) * Q_TILE]
            if np.all(blk < NEG_THRESH):
                cls[(ki, j)] = "skip"
            elif np.all(blk == 0.0):
                cls[(ki, j)] = "clean"
            else:
                cls[(ki, j)] = "partial"
                d = ki * K_CHUNK - j * Q_TILE
                assert 0 <= d <= Q_TILE - K_CHUNK, f"bad offset {d}"
                assert np.all(blk[:, :d] < NEG_THRESH)
                assert np.array_equal(blk[:, d:d + K_CHUNK], tri)
                assert np.all(blk[:, d + K_CHUNK:] == 0.0)
                d_off[(ki, j)] = d
    for j in range(S // Q_TILE):
        for ki in range(S // K_CHUNK):
            c = cls[(ki, j)]
            if c != "skip":
                assert d_off.get((ki, j), 0) == 0, "first block not full width"
                break
    return cls, d_off


def build_kernel(S, cls, d_off, repeats=1, loop=False, only=None):
    """Build the per-core SPMD Bass program (identical on all cores).

    repeats: body repetitions.  loop=False unrolls them; loop=True wraps
    the body in a tc.For_i hardware loop (small NEFF, used for timing).
    """
    n_k = S // K_CHUNK
    n_q = S // Q_TILE

    nc = bacc.Bacc("TRN2", target_bir_lowering=False, debug=False,
                   num_devices=N_CORES)

    xT_d = nc.dram_tensor("xT", [N_STATE, S], F32R, kind="ExternalInput")
    wq_d = nc.dram_tensor("wq", [N_STATE, E], F32R, kind="ExternalInput")
    wk_d = nc.dram_tensor("wk", [N_STATE, E], F32R, kind="ExternalInput")
    wv_d = nc.dram_tensor("wv", [N_STATE, E], F32R, kind="ExternalInput")
    wo_d = nc.dram_tensor("wo", [E, N_STATE], F32R, kind="ExternalInput")
    bq_d = nc.dram_tensor("bq", [E], F32, kind="ExternalInput")
    bv_d = nc.dram_tensor("bv", [E], F32, kind="ExternalInput")
    ident_d = nc.dram_tensor("ident", [128, 128], BF16, kind="ExternalInput")
    tri_d = nc.dram_tensor("tri", [K_CHUNK, K_CHUNK], BF16,
                           kind="ExternalInput")
    yT_d = nc.dram_tensor("yT", [N_STATE, S], F32, kind="ExternalOutput")

    with tile.TileContext(nc) as tc, ExitStack() as ctx:
        const = ctx.enter_context(tc.tile_pool(name="const", bufs=1))
        xpool = ctx.enter_context(tc.tile_pool(name="xpool", bufs=4))
        stage = ctx.enter_context(tc.tile_pool(name="stage", bufs=3))
        wexp = ctx.enter_context(tc.tile_pool(name="wexp", bufs=4))
        bcast = ctx.enter_context(tc.tile_pool(name="bcast", bufs=2))
        # PSUM: 8 banks exactly: unified unit ring 3x2 + ot 2x1
        psU = ctx.enter_context(tc.tile_pool(name="psU", bufs=3, space="PSUM"))
        psO = ctx.enter_context(tc.tile_pool(name="psO", bufs=2, space="PSUM"))

        # ---- resident constants / weights ----
        wq_sb = const.tile([128, N_D, E], F32R, tag="wq_sb")
        wk_sb = const.tile([128, N_D, E], F32R, tag="wk_sb")
        wv_sb = const.tile([128, N_D, E], F32R, tag="wv_sb")
        bq_sb = const.tile([128, HP], F32, tag="bq_sb")
        bv_sb = const.tile([128, HP], F32, tag="bv_sb")
        ident = const.tile([128, 128], BF16, tag="ident")
        tri_sb = const.tile([K_CHUNK, K_CHUNK], BF16, tag="tri_sb")
        wo_sb = const.tile([128, HP, N_STATE], F32R, tag="wo_sb")

        qTz = [[const.tile([128, S], BF16, name=f"qTz{g}{h}",
                           tag=f"qTz{g}{h}") for h in range(2)]
               for g in range(HP)]
        kT = [const.tile([128, S], BF16, name=f"kT{g}", tag=f"kT{g}")
              for g in range(HP)]
        onT = [const.tile([128, S], F32R, name=f"onT{g}", tag=f"onT{g}")
               for g in range(HP)]
        vn = [const.tile([128, n_k, 2 * (HD + 1)], BF16, name=f"vn{g}",
                         tag=f"vn{g}") for g in range(HP)]

        xts = [None] * n_q

        def load_x(j):
            ts = slice(j * Q_TILE, (j + 1) * Q_TILE)
            xt = xpool.tile([128, N_D, Q_TILE], F32R, tag="xt")
            src_ap = xT_d[:, ts].rearrange("(c p) t -> p c t", p=128)
            h = N_D // 2
            nc.sync.dma_start(xt[:, 0:h, :], src_ap[:, 0:h, :])
            nc.sync.dma_start(xt[:, h:N_D, :], src_ap[:, h:N_D, :])
            xts[j] = xt

        def load_weights():
            for w_sb, w_d in ((wq_sb, wq_d), (wk_sb, wk_d), (wv_sb, wv_d)):
                nc.sync.dma_start(
                    w_sb[:], w_d[:].rearrange("(c p) e -> p c e", p=128))
            nc.sync.dma_start(bq_sb[:],
                              bq_d[:].rearrange("(g p) -> p g", g=HP))
            nc.sync.dma_start(bv_sb[:],
                              bv_d[:].rearrange("(g p) -> p g", g=HP))
            nc.sync.dma_start(ident[:], ident_d[:])
            nc.sync.dma_start(tri_sb[:], tri_d[:])
            nc.sync.dma_start(wo_sb[:],
                              wo_d[:].rearrange("(g p) m -> p g m", p=128))

        def init_ones():
            for g in range(HP):
                ones_ap = vn[g][:].rearrange(
                    "p k (g2 c) -> p k g2 c", g2=2)[:, :, :, HD:HD + 1]
                nc.gpsimd.memset(ones_ap, 1.0)
                for h in range(2):
                    pad = slice((1 - h) * HD, (2 - h) * HD)
                    nc.gpsimd.memset(qTz[g][h][pad, :], 0.0)

        def stage_a_qk(j, g):
            """Q/K projections for token tile j, head-pair g."""
            ts = slice(j * Q_TILE, (j + 1) * Q_TILE)
            es = slice(g * 128, (g + 1) * 128)
            xt = xts[j]
            pk = psU.tile([128, 2, Q_TILE], F32, tag="u")
            for c in range(N_D):
                nc.tensor.matmul(pk[:, 0, :], wq_sb[:, c, es], xt[:, c, :],
                                 start=(c == 0), stop=(c == N_D - 1))
            for c in range(N_D):
                nc.tensor.matmul(pk[:, 1, :], wk_sb[:, c, es], xt[:, c, :],
                                 start=(c == 0), stop=(c == N_D - 1))
            for h in range(2):
                hs2 = slice(h * HD, (h + 1) * HD)
                nc.vector.tensor_scalar(qTz[g][h][hs2, ts], pk[hs2, 0, :],
                                        bq_sb[hs2, g:g + 1], None, ALU.add)
            nc.scalar.activation(kT[g][:, ts], pk[:, 1, :], AF.Copy)

        def stage_a_v(j, g):
            """V projection + transpose for token tile j, head-pair g."""
            es = slice(g * 128, (g + 1) * 128)
            xt = xts[j]
            pv = psU.tile([128, 2, Q_TILE], F32, tag="u")
            for c in range(N_D):
                nc.tensor.matmul(pv[:, 0, :], wv_sb[:, c, es], xt[:, c, :],
                                 start=(c == 0), stop=(c == N_D - 1))
            vs = stage.tile([128, Q_TILE], BF16, tag="vs")
            nc.scalar.activation(vs[:], pv[:, 0, :], AF.Identity,
                                 bias=bv_sb[:, g:g + 1])
            tp = pv[:, 1, 0:Q_TILE // 2].bitcast(BF16).rearrange(
                "p (k c) -> p k c", k=4)
            for c2 in range(4):
                nc.tensor.matmul(tp[:, c2, :], vs[:, c2 * 128:(c2 + 1) * 128],
                                 ident[:], is_transpose=True)
            in_ap = tp.rearrange("p k (g2 c) -> p k g2 c", g2=2)
            out_ap = vn[g][:, 4 * j:4 * j + 4, :].rearrange(
                "p k (g2 c) -> p k g2 c", g2=2)[:, :, :, 0:HD]
            nc.vector.tensor_copy(out_ap, in_ap)

        def stage_a_unit(j, g):
            stage_a_qk(j, g)
            stage_a_v(j, g)

        def stage_b(g, h, inject=None, inject_from_half=0):
            inject = list(inject or ())
            n_emit = max(1, (n_q // 2 - inject_from_half) * n_k)
            quota = max(1, -(-len(inject) // n_emit)) if inject else 0
            hs = slice(h * HD, (h + 1) * HD)
            vsl = slice(h * (HD + 1), (h + 1) * (HD + 1))
            for jp in range(n_q // 2):
                ot = [psO.tile([HD + 1, Q_TILE], F32, name=f"ot{jj}",
                               tag="ot") for jj in range(2)]
                first = [True, True]

                def finalize_j(jj):
                    j = 2 * jp + jj
                    qs = slice(j * Q_TILE, (j + 1) * Q_TILE)
                    rd = stage.tile([1, Q_TILE], F32, tag="rd")
                    nc.vector.reciprocal(rd[:], ot[jj][HD:HD + 1, :])
                    bc = bcast.tile([HD, Q_TILE], F32, tag="bc")
                    rdap = rd[:]
                    rd_rep = bass.AP(rdap.tensor, rdap.offset,
                                     [list(rdap.ap[0]), [0, HD],
                                      list(rdap.ap[1])])
                    nc.scalar.dma_start(bc[:], rd_rep)
                    oraw = stage.tile([HD, Q_TILE], F32, tag="oraw")
                    nc.vector.tensor_copy(oraw[:], ot[jj][0:HD, :])
                    nc.gpsimd.tensor_tensor(onT[g][hs, qs], oraw[:],
                                            bc[:], ALU.mult)

                def emit_o(ki, pair, ds, wt):
                    for jj in range(2):
                        if pair[jj] == "skip":
                            continue
                        j = 2 * jp + jj
                        d = ds[jj]
                        last = not any(cls[(k2, j)] != "skip"
                                       for k2 in range(ki + 1, n_k))
                        nc.tensor.matmul(
                            ot[jj][:, d:Q_TILE], vn[g][:, ki, vsl],
                            wt[:, jj, d:Q_TILE],
                            start=first[jj], stop=last, skip_group_check=True)
                        first[jj] = False
                        if last:
                            finalize_j(jj)

                pending = None
                for ki in range(n_k):
                    if jp >= inject_from_half:
                        for _ in range(quota):
                            if inject:
                                inject.pop(0)()
                    pair = (cls[(ki, 2 * jp)], cls[(ki, 2 * jp + 1)])
                    if pair == ("skip", "skip"):
                        continue
                    ks = slice(ki * K_CHUNK, (ki + 1) * K_CHUNK)
                    sc = psU.tile([128, 2, Q_TILE], F32, tag="u")
                    wt = wexp.tile([128, 2, Q_TILE], BF16, tag="wt")
                    ds = [0, 0]
                    for jj in range(2):
                        if pair[jj] == "skip":
                            continue
                        j = 2 * jp + jj
                        d = d_off.get((ki, j), 0)
                        ds[jj] = d
                        qs = slice(j * Q_TILE + d, (j + 1) * Q_TILE)
                        nc.tensor.matmul(sc[:, jj, d:Q_TILE], kT[g][:, ks],
                                         qTz[g][h][:, qs],
                                         start=True, stop=True)
                    if pending is not None:
                        emit_o(*pending)
                    if pair == ("clean", "clean"):
                        nc.scalar.activation(wt[:, :, :], sc[:, :, :], AF.Exp)
                    else:
                        for jj in range(2):
                            if pair[jj] == "skip":
                                continue
                            d = ds[jj]
                            nc.scalar.activation(wt[:, jj, d:Q_TILE],
                                                 sc[:, jj, d:Q_TILE], AF.Exp)
                    for jj in range(2):
                        if pair[jj] == "partial":
                            d = ds[jj]
                            reg = wt[:, jj, d:d + K_CHUNK]
                            nc.vector.tensor_tensor(reg, reg, tri_sb[:],
                                                    ALU.mult)
                    pending = (ki, pair, ds, wt)
                if pending is not None:
                    emit_o(*pending)
            while inject:
                inject.pop(0)()

        c_alt = [0]

        def stage_c_unit(m, jp, tail=False):
            ms = slice(m * 128, (m + 1) * 128)
            yp = psU.tile([128, 2, Q_TILE], F32, tag="u")
            for jj in range(2):
                qs = slice((2 * jp + jj) * Q_TILE, (2 * jp + jj + 1) * Q_TILE)
                nc.tensor.matmul(yp[:, jj, :], wo_sb[:, 0, ms], onT[0][:, qs],
                                 start=True, stop=False)
                nc.tensor.matmul(yp[:, jj, :], wo_sb[:, 1, ms], onT[1][:, qs],
                                 start=False, stop=True)
            ys = stage.tile([128, 2, Q_TILE], F32, tag="ys", bufs=2)
            if c_alt[0] % 2 == 0:
                nc.vector.tensor_copy(ys[:], yp[:])
            else:
                nc.scalar.activation(ys[:], yp[:], AF.Copy)
            c_alt[0] += 1
            nc.sync.dma_start(
                yT_d[ms, 2 * jp * Q_TILE:(2 * jp + 2) * Q_TILE],
                ys[:].rearrange("p a q -> p (a q)"))

        def unit_thunks_a(g):
            out = []
            for j in range(n_q):
                out.append((lambda j_, g_: (lambda: stage_a_qk(j_, g_)))(j, g))
                out.append((lambda j_, g_: (lambda: stage_a_v(j_, g_)))(j, g))
            return out

        def unit_thunks_c(jp):
            return [(lambda m_, jp_: (lambda: stage_c_unit(m_, jp_)))(m, jp)
                    for m in range(N_STATE // 128)]

        def body():
            for j in range(n_q):
                load_x(j)
            stage_a_unit(0, 0)
            stage_a_unit(1, 0)
            a1 = unit_thunks_a(0)
            a2 = unit_thunks_a(1)
            stage_b(0, 0, inject=a1[4:8])
            stage_b(0, 1, inject=a2[0:4])
            stage_b(1, 0, inject=a2[4:8])
            stage_b(1, 1, inject=unit_thunks_c(0), inject_from_half=1)
            for m in range(N_STATE // 128):
                stage_c_unit(m, 1, tail=True)

        def body_only_a():
            for j in range(n_q):
                load_x(j)
                for g in range(HP):
                    stage_a_unit(j, g)

        def body_only_b():
            for g in range(HP):
                for h in range(2):
                    stage_b(g, h)

        def body_only_c():
            for jp in range(n_q // 2):
                for m in range(N_STATE // 128):
                    stage_c_unit(m, jp, tail=True)

        def body_only_e():
            # Act-paced micro: scores matmul pair + pair exp, 64 units
            for u in range(64):
                ki = u % 8
                ks = slice(ki * K_CHUNK, (ki + 1) * K_CHUNK)
                sc = psU.tile([128, 2, Q_TILE], F32, tag="u")
                wt = wexp.tile([128, 2, Q_TILE], BF16, tag="wt")
                for jj in range(2):
                    qs = slice(jj * Q_TILE, (jj + 1) * Q_TILE)
                    nc.tensor.matmul(sc[:, jj, :], kT[0][:, ks],
                                     qTz[0][0][:, qs], start=True, stop=True)
                nc.scalar.activation(wt[:, :, :], sc[:, :, :], AF.Exp)

        def body_only_m():
            # PE-paced micro: 16 chains x 8 proj-style matmuls (K=128, f32r)
            for u in range(16):
                ps = psU.tile([128, 2, Q_TILE], F32, tag="u")
                for c in range(N_D):
                    nc.tensor.matmul(ps[:, 0, :], wq_sb[:, c, 0:128],
                                     xts[0][:, c, :],
                                     start=(c == 0), stop=(c == N_D - 1))

        def body_only_p():
            # scores-style matmuls (K=64) with bf16 operands
            kb = const.tile([HD, S // 4], BF16, tag="kb")
            qb = const.tile([HD, S // 4], BF16, tag="qb")
            nc.scalar.activation(kb[:], kT[0][0:HD, 0:S // 4], AF.Copy)
            nc.scalar.activation(qb[:], qTz[0][0][0:HD, 0:S // 4], AF.Copy)
            for u in range(64):
                ki = u % 4
                ks = slice(ki * K_CHUNK, (ki + 1) * K_CHUNK)
                sc = psU.tile([128, 2, Q_TILE], F32, tag="u")
                for jj in range(2):
                    nc.tensor.matmul(sc[:, jj, :], kb[:, ks],
                                     qb[:, 0:Q_TILE], start=True, stop=True)

        def body_only_o():
            # o-style matmuls: bf16 stationary vn, bf16 moving, K=128
            vsm = stage.tile([128, Q_TILE], BF16, tag="vs")
            nc.vector.tensor_copy(vsm[:], kT[0][:, 0:Q_TILE])
            for u in range(128):
                ki = u % 8
                po = psU.tile([128, 2, Q_TILE], F32, tag="u")
                nc.tensor.matmul(po[:, 0, 0:Q_TILE].rearrange(
                    "p q -> p q")[0:HD + 1, :], vn[0][:, ki, 0:HD + 1],
                    vsm[:], start=True, stop=True)

        def body_only_n():
            # PE-paced micro: 128 scores-style matmuls (K=64, f32r)
            for u in range(64):
                ki = u % 8
                ks = slice(ki * K_CHUNK, (ki + 1) * K_CHUNK)
                sc = psU.tile([128, 2, Q_TILE], F32, tag="u")
                for jj in range(2):
                    qs = slice(jj * Q_TILE, (jj + 1) * Q_TILE)
                    nc.tensor.matmul(sc[:, jj, :], kT[0][:, ks],
                                     qTz[0][1][:, qs], start=True, stop=True)

        # ---- emission ----
        load_weights()
        init_ones()

        if only in ("B", "C", "E", "N", "P", "O"):
            for j in range(n_q):
                load_x(j)
                for g in range(HP):
                    stage_a_unit(j, g)
        if only == "M":
            load_x(0)
        if only == "C":
            for g in range(HP):
                for h in range(2):
                    stage_b(g, h)

        run_body = {None: body, "A": body_only_a, "B": body_only_b,
                    "C": body_only_c, "E": body_only_e, "M": body_only_m,
                    "N": body_only_n, "P": body_only_p, "O": body_only_o}[only]

        if loop and repeats > 1:
            with tc.For_i(0, repeats):
                run_body()
        else:
            for _ in range(repeats):
                run_body()

    nc.finalize()
    return nc


def _prep(x, mask, Wq, bq, Wk, Wv, bv, Wo):
    """Host-side layout prep + slicing; returns (maskT, in_maps)."""
    x = np.asarray(x, np.float32)
    maskT = np.ascontiguousarray(np.asarray(mask, np.float32).T)
    xT = np.ascontiguousarray(x.transpose(0, 2, 1))
    Wq = np.asarray(Wq, np.float32) * SCALE
    Wk = np.asarray(Wk, np.float32) * SCALE
    Wv = np.asarray(Wv, np.float32)
    Wo = np.asarray(Wo, np.float32)
    bq = np.asarray(bq, np.float32) * SCALE
    bv = np.asarray(bv, np.float32)
    import ml_dtypes
    tri = np.ascontiguousarray(
        (maskT[0:K_CHUNK, 0:K_CHUNK] == 0.0).astype(ml_dtypes.bfloat16))
    ident_b = np.eye(128, dtype=ml_dtypes.bfloat16)
    in_maps = []
    for c in range(N_CORES):
        b, hq = c // 4, c % 4
        cs = slice(hq * E, (hq + 1) * E)
        in_maps.append({
            "xT": xT[b],
            "wq": np.ascontiguousarray(Wq[:, cs]),
            "wk": np.ascontiguousarray(Wk[:, cs]),
            "wv": np.ascontiguousarray(Wv[:, cs]),
            "wo": np.ascontiguousarray(Wo[cs, :]),
            "bq": np.ascontiguousarray(bq[cs]),
            "bv": np.ascontiguousarray(bv[cs]),
            "ident": ident_b,
            "tri": tri,
        })
    return maskT, in_maps


_NC_CACHE = {}


def _get_nc(S, cls_key, cls, d_off, repeats=1, loop=False, only=None):
    key = (S, cls_key, repeats, loop, only)
    if key not in _NC_CACHE:
        _NC_CACHE[key] = build_kernel(S, cls, d_off, repeats=repeats,
                                      loop=loop, only=only)
    return _NC_CACHE[key]


def _gather(results, B, S, bo):
    acc = np.zeros((B, N_STATE, S), dtype=np.float64)
    for c, r in enumerate(results):
        acc[c // 4] += r["yT"].astype(np.float64)
    return (acc.transpose(0, 2, 1).astype(np.float32)
            + np.asarray(bo, np.float32))


def run(x, mask, Wq, bq, Wk, Wv, bv, Wo, bo, trace=False):
    B, S, D = x.shape
    maskT, in_maps = _prep(x, mask, Wq, bq, Wk, Wv, bv, Wo)
    cls, d_off = classify_blocks(maskT)
    cls_key = hash(tuple(sorted((k, v) for k, v in cls.items())))
    nc = _get_nc(S, cls_key, cls, d_off)
    res = bass_utils.run_bass_kernel_spmd(
        nc, in_maps, core_ids=list(range(N_CORES)), trace=trace)
    return _gather(res.results, B, S, bo), res


def kernel(x, mask, Wq, bq, Wk, Wv, bv, Wo, bo):
    y, _ = run(x, mask, Wq, bq, Wk, Wv, bv, Wo, bo, trace=False)
    return y


def time_run(x, mask, Wq, bq, Wk, Wv, bv, Wo, bo, iters=20, repeats=1,
             loop=False, only=None):
    """Measure per-iteration device execution time of the SPMD program.

    Mirrors bass2jax.run_bass_via_pjrt's multi-core lowering, but keeps
    inputs device-resident and chains donated output buffers so `iters`
    executions pipeline back-to-back; returns (y, seconds_per_iter).
    """
    import time as _time
    import jax
    from jax.experimental.shard_map import shard_map
    from jax.sharding import Mesh, NamedSharding, PartitionSpec
    from concourse import bass2jax
    from concourse.bass2jax import _bass_exec_p, install_neuronx_cc_hook

    install_neuronx_cc_hook()
    B, S, D = x.shape
    maskT, in_maps = _prep(x, mask, Wq, bq, Wk, Wv, bv, Wo)
    cls, d_off = classify_blocks(maskT)
    cls_key = hash(tuple(sorted((k, v) for k, v in cls.items())))
    nc = _get_nc(S, cls_key, cls, d_off, repeats=repeats, loop=loop,
                 only=only)

    in_names, out_names, out_avals, zero_outs = [], [], [], []
    partition_name = (nc.partition_id_tensor.name
                      if nc.partition_id_tensor else None)
    for alloc in nc.m.functions[0].allocations:
        if not isinstance(alloc, mybir.MemoryLocationSet):
            continue
        name = alloc.memorylocations[0].name
        if alloc.kind == "ExternalInput":
            if name != partition_name:
                in_names.append(name)
        elif alloc.kind == "ExternalOutput":
            out_names.append(name)
            shape = tuple(alloc.tensor_shape)
            dtype = mybir.dt.np(alloc.dtype)
            out_avals.append((shape, dtype))
            zero_outs.append(np.zeros(shape, dtype))
    n_params = len(in_names)
    n_outs = len(out_names)
    all_in_names = list(in_names) + list(out_names)
    if partition_name is not None:
        all_in_names.append(partition_name)

    def _body(*args):
        operands = list(args)
        if partition_name is not None:
            operands.append(bass2jax.partition_id_tensor())
        outs = _bass_exec_p.bind(
            *operands,
            out_avals=tuple(
                jax.core.ShapedArray(s, d) for s, d in out_avals),
            in_names=tuple(all_in_names),
            out_names=tuple(out_names),
            lowering_input_output_aliases=(),
            sim_require_finite=True,
            sim_require_nnan=True,
            nc=nc,
        )
        return tuple(outs)

    devices = jax.devices()[:N_CORES]
    mesh = Mesh(np.asarray(devices), ("core",))
    spec = PartitionSpec("core")
    donate = tuple(range(n_params, n_params + n_outs))
    sharded = jax.jit(
        shard_map(_body, mesh=mesh, in_specs=(spec,) * (n_params + n_outs),
                  out_specs=(spec,) * n_outs, check_rep=False),
        donate_argnums=donate, keep_unused=True)

    sh = NamedSharding(mesh, spec)
    dev_in = [
        jax.device_put(
            np.concatenate([np.asarray(in_maps[c][nm]) for c in range(N_CORES)],
                           axis=0), sh)
        for nm in in_names
    ]
    out = sharded(*dev_in, *[
        jax.device_put(np.zeros((N_CORES * z.shape[0], *z.shape[1:]), z.dtype),
                       sh) for z in zero_outs])
    jax.block_until_ready(out)  # warmup + compile
    t0 = _time.perf_counter()
    for _ in range(iters):
        out = sharded(*dev_in, *out)
    jax.block_until_ready(out)
    dt = (_time.perf_counter() - t0) / iters

    yT_all = np.asarray(out[out_names.index("yT")])
    results = [{"yT": yT_all.reshape(N_CORES, N_STATE, S)[c]}
               for c in range(N_CORES)]
    y = _gather(results, B, S, bo)
    return y, dt
